# revision 1
# baseline (speedup 1.0000x reference)
"""Trainium2 Bass kernel for nn_ODE_71743133713072.

Semantics (unrolled from the reference lax.scan):
  out[:, 0]   = lat[:, 0]
  out[:, t+1] = lat[:, t] + dt_eff[t] * f(lat[:, t])   for t = 0..99
                (dt_eff[1] = 0 reproduces the scan's zero-length first gap)
  y = out[:, 100]
  out[:, k+1] = y = y + h * f(y)                        for k = 100..118
where f is the D->U->U->D tanh MLP and all nonzero dt equal h = ts[1]-ts[0]
(linspace; per-step fp32 diffs differ from h by <=1 ulp, far below the bf16
matmul noise floor, so h is folded into W3/b3 on the host).

Sharding: batch 1024 over 8 cores (128 rows/core, exactly the partition
width). Matmuls in bf16 with fp32 PSUM accumulation. Layers 1-2 run
feature-on-partition (weights stationary); layer 3 swaps roles (activations
stationary, h*W3 moving) so its output lands in natural row layout and the
Euler update is a single PSUM+SBUF add. b3*h enters layer 3's accumulation
group via a K=1 ones-row matmul.
"""

import os
import sys
from contextlib import ExitStack

import numpy as np

for _p in ("/opt/trn_rl_repo", "/root/.axon_site/_ro/trn_rl_repo"):
    if os.path.isdir(_p) and _p not in sys.path:
        sys.path.append(_p)

import ml_dtypes  # noqa: E402

B, T_OBS, KPRED, D = 1024, 100, 20, 256
T = T_OBS + KPRED          # 120
NCORES = 8
PB = B // NCORES           # 128 rows per core
P = 128
G = 4                      # time steps per compute group
NG = T_OBS // G            # 25 groups


def _emit(ctx, tc, lat, w3hd, w8d, bpk, br8p, id8d, id32d, ones8d, out, h):
    import concourse.mybir as mybir

    nc = tc.nc
    F32 = mybir.dt.float32
    BF16 = mybir.dt.bfloat16
    FP8 = mybir.dt.float8e4
    AF = mybir.ActivationFunctionType
    ALU = mybir.AluOpType
    DR = mybir.MatmulPerfMode.DoubleRow

    const = ctx.enter_context(tc.tile_pool(name="const", bufs=1))
    w3sb = const.tile([P, 2, D], BF16, tag="w3")
    for kc in range(2):
        nc.sync.dma_start(w3sb[:, kc, :], w3hd[kc * P:(kc + 1) * P, :])
    bsb = const.tile([P, 6], F32, tag="bias")
    nc.sync.dma_start(bsb[:], bpk[:])
    ones8 = const.tile([1, P], FP8, tag="ones8")
    nc.sync.dma_start(ones8[:], ones8d[:])
    # fp8 weights (x8-scaled): [P, 3(w), 2(kc), D]
    w8sb = const.tile([P, 3, 2, D], FP8, tag="w8")
    for wi in range(3):
        for kc in range(2):
            nc.sync.dma_start(w8sb[:, wi, kc, :], w8d[wi, kc * P:(kc + 1) * P, :])
    br8sb = const.tile([1, 2 * D], FP8, tag="br8")
    nc.sync.dma_start(br8sb[:], br8p[:])
    id16 = const.tile([P, P], BF16, tag="id16")
    nc.sync.dma_start(id16[:], id8d[:])
    id32 = const.tile([P, P], F32, tag="id32")
    nc.sync.dma_start(id32[:], id32d[:])

    b1ap = [bsb[:, 0:1], bsb[:, 1:2]]
    b2ap = [bsb[:, 2:3], bsb[:, 3:4]]
    b3hap = [bsb[:, 4:5], bsb[:, 5:6]]

    x32p = ctx.enter_context(tc.tile_pool(name="x32", bufs=4))
    x8p = ctx.enter_context(tc.tile_pool(name="x8", bufs=4))
    xtsbp = ctx.enter_context(tc.tile_pool(name="xtsb", bufs=3))
    hsbp = ctx.enter_context(tc.tile_pool(name="hsb", bufs=4))
    outp = ctx.enter_context(tc.tile_pool(name="outsb", bufs=4))
    chsb = ctx.enter_context(tc.tile_pool(name="chsb", bufs=3))

    xtps = ctx.enter_context(tc.tile_pool(name="xtps", bufs=1, space="PSUM"))
    mmps = ctx.enter_context(tc.tile_pool(name="mmps", bufs=2, space="PSUM"))
    fnps = ctx.enter_context(tc.tile_pool(name="fnps", bufs=2, space="PSUM"))
    chps = ctx.enter_context(tc.tile_pool(name="chps", bufs=1, space="PSUM"))

    h8 = float(h / 8.0)

    def stage_load(g):
        """load + cast + transpose + evac for one group; returns tiles."""
        t0 = g * G
        x32 = x32p.tile([P, G, D], F32, tag="x32")
        nc.sync.dma_start(x32[:], lat[:, t0:t0 + G, :])
        x16 = x8p.tile([P, G, D], BF16, tag="x16")
        nc.gpsimd.tensor_copy(x16[:], x32[:])
        xt = xtps.tile([P, 2, G * P], BF16, tag="xt")
        for tt in range(G):
            for dc in range(2):
                nc.tensor.transpose(
                    xt[:, dc, tt * P:(tt + 1) * P],
                    x16[:, tt, dc * P:(dc + 1) * P], id16[:])
        xts = xtsbp.tile([P, 2, G * P], FP8, tag="xts")
        for dc in range(2):
            nc.vector.tensor_copy(xts[:, dc, :], xt[:, dc, :])
        return x32, xts

    def stage_mlp(pair):
        """L1/L2 for a pair of groups with shared weight loads."""
        h1s = {}
        mm = {}
        for g, (x32, xts) in pair.items():
            mm[g] = mmps.tile([P, 2, G * P], F32, tag="mm", name="mm")
        for mc in range(2):
            for g in pair:
                nc.tensor.matmul(mm[g][:, mc, :],
                                 w8sb[:, 0, :, mc * P:(mc + 1) * P],
                                 pair[g][1][:], start=True, stop=True,
                                 perf_mode=DR)
        for g in pair:
            t = hsbp.tile([P, 2, G * P], FP8, tag="h1")
            for mc in range(2):
                nc.scalar.activation(t[:, mc, :], mm[g][:, mc, :], AF.Tanh,
                                     bias=b1ap[mc], scale=0.125)
            h1s[g] = t
        mm2 = {}
        for g in pair:
            mm2[g] = mmps.tile([P, 2, G * P], F32, tag="mm", name="mm2")
        for mc in range(2):
            for g in pair:
                nc.tensor.matmul(mm2[g][:, mc, :],
                                 w8sb[:, 1, :, mc * P:(mc + 1) * P],
                                 h1s[g][:], start=True, stop=True,
                                 perf_mode=DR)
        h2s = {}
        for g in pair:
            t = hsbp.tile([P, 2, G * P], FP8, tag="h2")
            for mc in range(2):
                nc.scalar.activation(t[:, mc, :], mm2[g][:, mc, :], AF.Tanh,
                                     bias=b2ap[mc], scale=0.125)
            h2s[g] = t
        return h2s

    def stage_out(g, x32, h2s_g):
        """L3 (role-swapped, fp8 DR) + Euler add + store for one group."""
        t0 = g * G
        o32 = outp.tile([P, G, D], F32, tag="o32")
        for half in range(2):
            fn = fnps.tile([P, 2, D], F32, tag="fn")
            # seed each subtile with 8*b3 broadcast (K=1 ones row)
            for i, tt in enumerate((2 * half, 2 * half + 1)):
                nc.tensor.matmul(fn[:, i, :], ones8[:], br8sb[:, 0:D],
                                 start=True, stop=False)
                nc.tensor.matmul(fn[:, i, :],
                                 h2s_g[:, :, tt * P:(tt + 1) * P],
                                 w8sb[:, 2, :, :],
                                 start=False, stop=True, perf_mode=DR)
            if g == 0 and half == 0:
                # t=0: normal Euler step; t=1: dt=0 -> out[:,2] = lat[:,1]
                nc.vector.scalar_tensor_tensor(
                    o32[:, 0, :], fn[:, 0, :], h8, x32[:, 0, :],
                    ALU.mult, ALU.add)
                nc.vector.tensor_copy(o32[:, 1, :], x32[:, 1, :])
            else:
                nc.vector.scalar_tensor_tensor(
                    o32[:, 2 * half:2 * half + 2, :].rearrange("p a b -> p (a b)"),
                    fn.rearrange("p a b -> p (a b)"), h8,
                    x32[:, 2 * half:2 * half + 2, :].rearrange("p a b -> p (a b)"),
                    ALU.mult, ALU.add)
        nc.sync.dma_start(out[:, t0 + 1:t0 + G + 1, :], o32[:])
        return o32

    def do_pair(ga, gb):
        pair = {}
        for g in (ga, gb):
            if g is not None:
                pair[g] = stage_load(g)
        h2s = stage_mlp(pair)
        outs = {}
        for g in pair:
            outs[g] = stage_out(g, pair[g][0], h2s[g])
        return outs

    def chain(o32_24):
        # y0 = out[:, 100] = o32_24[:, 3, :]; chain state transposed fp32.
        y0p = chps.tile([P, 2, P], F32, tag="ch")
        for dc in range(2):
            nc.tensor.transpose(y0p[:, dc, :],
                                o32_24[:, G - 1, dc * P:(dc + 1) * P], id32[:])
        yt = chsb.tile([P, 2, P], F32, tag="yt")
        nc.vector.tensor_copy(yt[:], y0p[:])

        for k in range(T_OBS, T - 1):
            y8 = chsb.tile([P, 2, P], FP8, tag="y8")
            nc.vector.tensor_copy(y8[:], yt[:])
            c1 = chps.tile([P, 2, P], F32, tag="ch")
            for mc in range(2):
                nc.tensor.matmul(c1[:, mc, :],
                                 w8sb[:, 0, :, mc * P:(mc + 1) * P],
                                 y8[:], start=True, stop=True, perf_mode=DR)
            c1s = chsb.tile([P, 2, P], FP8, tag="c1s")
            for mc in range(2):
                nc.scalar.activation(c1s[:, mc, :], c1[:, mc, :], AF.Tanh,
                                     bias=b1ap[mc], scale=0.125)
            c2 = chps.tile([P, 2, P], F32, tag="ch")
            for mc in range(2):
                nc.tensor.matmul(c2[:, mc, :],
                                 w8sb[:, 1, :, mc * P:(mc + 1) * P],
                                 c1s[:], start=True, stop=True, perf_mode=DR)
            c2s = chsb.tile([P, 2, P], BF16, tag="c2s")
            for mc in range(2):
                nc.scalar.activation(c2s[:, mc, :], c2[:, mc, :], AF.Tanh,
                                     bias=b2ap[mc], scale=0.125)
            # L3 in bf16 (w3sb = h*W3); b3*h joins in the update op below.
            c3 = chps.tile([P, 2, P], F32, tag="ch")
            for mc in range(2):
                for kc in range(2):
                    nc.tensor.matmul(c3[:, mc, :],
                                     w3sb[:, kc, mc * P:(mc + 1) * P],
                                     c2s[:, kc, :], start=(kc == 0),
                                     stop=(kc == 1))
            ytn = chsb.tile([P, 2, P], F32, tag="yt")
            for dc in range(2):
                nc.vector.scalar_tensor_tensor(
                    ytn[:, dc, :], c3[:, dc, :], b3hap[dc], yt[:, dc, :],
                    ALU.add, ALU.add)
            yt = ytn

            ynp = chps.tile([P, D], F32, tag="ch")
            for dc in range(2):
                nc.tensor.transpose(ynp[:, dc * P:(dc + 1) * P], yt[:, dc, :], id32[:])
            yns = chsb.tile([P, D], F32, tag="yns")
            nc.vector.tensor_copy(yns[:], ynp[:])
            nc.sync.dma_start(out[:, k + 1, :], yns[:])

    outs = do_pair(NG - 1, NG - 2)
    chain(outs[NG - 1])
    for p in range(0, NG - 2, 2):
        ga = p
        gb = p + 1 if p + 1 < NG - 2 else None
        do_pair(ga, gb)
    nc.sync.dma_start(out[:, 0, :], lat[:, 0, :])


def _build(h):
    import concourse.mybir as mybir
    import concourse.tile as tile
    from concourse import bacc

    F32 = mybir.dt.float32
    BF16 = mybir.dt.bfloat16
    FP8 = mybir.dt.float8e4

    nc = bacc.Bacc("TRN2", target_bir_lowering=False, debug=False,
                   num_devices=NCORES)
    lat = nc.dram_tensor("lat", [PB, T_OBS, D], F32, kind="ExternalInput").ap()
    w3hd = nc.dram_tensor("w3h", [D, D], BF16, kind="ExternalInput").ap()
    w8d = nc.dram_tensor("w8", [3, D, D], FP8, kind="ExternalInput").ap()
    bpk = nc.dram_tensor("bpack", [P, 6], F32, kind="ExternalInput").ap()
    br8p = nc.dram_tensor("brows8", [1, 2 * D], FP8, kind="ExternalInput").ap()
    id8d = nc.dram_tensor("id8", [P, P], BF16, kind="ExternalInput").ap()
    id32d = nc.dram_tensor("id32", [P, P], F32, kind="ExternalInput").ap()
    ones8d = nc.dram_tensor("ones8", [1, P], FP8, kind="ExternalInput").ap()
    out = nc.dram_tensor("out", [PB, T, D], F32, kind="ExternalOutput").ap()

    with tile.TileContext(nc) as tc, ExitStack() as ctx:
        _emit(ctx, tc, lat, w3hd, w8d, bpk, br8p, id8d, id32d, ones8d, out, h)
    nc.compile()
    return nc


def _host_inputs(inputs):
    ts = np.asarray(inputs["time_steps"], np.float32)
    h = float(np.float32(ts[1]) - np.float32(ts[0]))

    bf = ml_dtypes.bfloat16
    f8 = ml_dtypes.float8_e4m3
    w3h = (np.asarray(inputs["W3"], np.float32) * np.float32(h)).astype(bf)
    b1 = np.asarray(inputs["b1"], np.float32)
    b2 = np.asarray(inputs["b2"], np.float32)
    b3h = np.asarray(inputs["b3"], np.float32) * np.float32(h)
    bpack = np.stack([b1[:P], b1[P:], b2[:P], b2[P:], b3h[:P], b3h[P:]],
                     axis=1).astype(np.float32)
    w8 = np.stack([
        (8.0 * np.asarray(inputs["W1"], np.float32)),
        (8.0 * np.asarray(inputs["W2"], np.float32)),
        (8.0 * np.asarray(inputs["W3"], np.float32)),
    ]).astype(f8)
    b3s8 = (8.0 * np.asarray(inputs["b3"], np.float32))
    brows8 = np.concatenate([b3s8, b3s8]).reshape(1, 2 * D).astype(f8)
    id8 = np.eye(P, dtype=np.float32).astype(bf)
    id32 = np.eye(P, dtype=np.float32)
    ones8 = np.ones((1, P), np.float32).astype(f8)

    shared = dict(w3h=w3h, w8=w8, bpack=bpack, brows8=brows8,
                  id8=id8, id32=id32, ones8=ones8)
    return h, shared


_CACHE = {}


def kernel(**inputs):
    from concourse.bass_utils import run_bass_kernel_spmd

    lat_full = np.ascontiguousarray(np.asarray(inputs["latents"], np.float32))
    h, shared = _host_inputs(inputs)

    if h not in _CACHE:
        _CACHE[h] = _build(h)
    nc = _CACHE[h]

    in_maps = []
    for c in range(NCORES):
        m = dict(shared)
        m["lat"] = np.ascontiguousarray(lat_full[c * PB:(c + 1) * PB])
        in_maps.append(m)
    res = run_bass_kernel_spmd(nc, in_maps, list(range(NCORES)))
    outs = [res.results[c]["out"] for c in range(NCORES)]
    return np.concatenate(outs, axis=0)



# revision 9
# speedup vs baseline: 1.2729x; 1.2729x over previous
"""Trainium2 Bass kernel for nn_ODE_71743133713072 (v2).

Semantics (unrolled from the reference lax.scan; time_steps = linspace, all
nonzero gaps equal h = ts[1]-ts[0]):
  out[:, 0]   = lat[:, 0]
  out[:, 2]   = lat[:, 1]                     (the scan's zero-length gap)
  out[:, t+1] = lat[:, t] + h * f(lat[:, t])  for t = 0..99, t != 1
  y = out[:, 100]
  out[:, k+1] = y = y + h * f(y)              for k = 100..118
where f is the D->U->U->D tanh MLP.

v2 layout strategy: everything on device lives FEATURE-ON-PARTITION
(transposed). The host pre-transposes the latents into
  xT8  [128p, 2dc, 100t, 128b] fp8   (matmul operand, unscaled)
  xT16 [128p, 2dc, 100t, 128b] fp16  (Euler base, with h*b3 pre-added)
and the device writes the output transposed (oT [128p, 2dc, 120t, 128b]
f32); the host de-transposes after gather. This removes every PE
transpose, every on-device cast, and every bias-seed matmul from the
parallel part: biases b1/b2 ride along as per-partition act biases, and
b3*h is folded into xT16. Matmuls are fp8 DoubleRow (K=256 in one pass,
0.5 cycles/col); weight loads pipeline behind the previous matmul.

The 19-step prediction chain keeps a transposed f32 carry, uses fp8-DR
for all three layers with tiny K=2 PSUM bias seeds, and is interleaved
between the parallel groups so its serial latency hides behind
DMA-bound group work.
"""

import os
import sys
from contextlib import ExitStack

import numpy as np

for _p in ("/opt/trn_rl_repo", "/root/.axon_site/_ro/trn_rl_repo"):
    if os.path.isdir(_p) and _p not in sys.path:
        sys.path.append(_p)

import ml_dtypes  # noqa: E402

B, T_OBS, KPRED, D = 1024, 100, 20, 256
T = T_OBS + KPRED          # 120
NCORES = 8
PB = B // NCORES           # 128 rows per core
P = 128
W = 8                      # frames per full group
NG_FULL = 12               # 12 full groups of 8 = 96 frames
W_LAST = 4                 # +1 group of 4 (frames 96..99)
NSTEPS = T - 1 - T_OBS     # 19 chain steps


def _emit(ctx, tc, xT8d, xT16d, w8d, bactd, bseedd, ones8d, oTd, h):
    import concourse.mybir as mybir

    nc = tc.nc
    F32 = mybir.dt.float32
    FP16 = mybir.dt.float16
    FP8 = mybir.dt.float8e4
    AF = mybir.ActivationFunctionType
    ALU = mybir.AluOpType
    DR = mybir.MatmulPerfMode.DoubleRow

    h8 = float(h / 8.0)

    const = ctx.enter_context(tc.tile_pool(name="const", bufs=1))
    w8 = const.tile([P, 3, 2, D], FP8, tag="w8")
    nc.sync.dma_start(w8[:], w8d[:])
    bact = const.tile([P, 4], F32, tag="bact")
    nc.sync.dma_start(bact[:], bactd[:])
    bseed = const.tile([1, 2, 3, D], FP8, tag="bseed")
    nc.sync.dma_start(bseed[:], bseedd[:])
    ones8 = const.tile([1, 2, P], FP8, tag="ones8")
    nc.sync.dma_start(ones8[:], ones8d[:])

    x16p = ctx.enter_context(tc.tile_pool(name="x16", bufs=3))
    x8p = ctx.enter_context(tc.tile_pool(name="x8", bufs=3))
    hp = ctx.enter_context(tc.tile_pool(name="hact", bufs=4))
    oTp = ctx.enter_context(tc.tile_pool(name="oT", bufs=3))
    psp = ctx.enter_context(tc.tile_pool(name="ps", bufs=3, space="PSUM"))

    y8p = ctx.enter_context(tc.tile_pool(name="y8", bufs=2))
    hcp = ctx.enter_context(tc.tile_pool(name="hc", bufs=4))
    collp = ctx.enter_context(tc.tile_pool(name="coll", bufs=2))
    chps = ctx.enter_context(tc.tile_pool(name="chps", bufs=2, space="PSUM"))

    def group(g):
        """One parallel group: frames t0..t0+w-1 -> oT[:, :, t0+1:t0+w+1]."""
        w = W if g < NG_FULL else W_LAST
        t0 = g * W
        nw = w * P
        x16 = x16p.tile([P, 2, W, P], FP16, tag="x16")
        nc.sync.dma_start(x16[:, :, 0:w, :], xT16d[:, :, t0:t0 + w, :])
        x8 = x8p.tile([P, 2, W, P], FP8, tag="x8")
        nc.sync.dma_start(x8[:, :, 0:w, :], xT8d[:, :, t0:t0 + w, :])

        h1 = hp.tile([P, 2, W * P], FP8, tag="h1")
        h2 = hp.tile([P, 2, W * P], FP8, tag="h2")
        oT = oTp.tile([P, 2, W, P], F32, tag="oT")

        mm1 = [psp.tile([P, W * P], F32, tag="ps", name="mm1") for _ in range(2)]
        for mc in range(2):
            for q in range(w // 4):
                nc.tensor.matmul(
                    mm1[mc][:, q * 512:(q + 1) * 512],
                    w8[:, 0, :, mc * P:(mc + 1) * P],
                    x8[:, :, 4 * q:4 * q + 4, :], start=True, stop=True,
                    perf_mode=DR)
        for mc in range(2):
            nc.scalar.activation(h1[:, mc, 0:nw], mm1[mc][:, 0:nw], AF.Tanh,
                                 bias=bact[:, mc:mc + 1], scale=0.125)
        mm2 = [psp.tile([P, W * P], F32, tag="ps", name="mm2") for _ in range(2)]
        for mc in range(2):
            for q in range(w // 4):
                nc.tensor.matmul(
                    mm2[mc][:, q * 512:(q + 1) * 512],
                    w8[:, 1, :, mc * P:(mc + 1) * P],
                    h1[:, :, q * 512:(q + 1) * 512], start=True, stop=True,
                    perf_mode=DR)
        for mc in range(2):
            nc.scalar.activation(h2[:, mc, 0:nw], mm2[mc][:, 0:nw], AF.Tanh,
                                 bias=bact[:, 2 + mc:3 + mc], scale=0.125)
        fT = [psp.tile([P, W * P], F32, tag="ps", name="fT") for _ in range(2)]
        for mc in range(2):
            for q in range(w // 4):
                nc.tensor.matmul(
                    fT[mc][:, q * 512:(q + 1) * 512],
                    w8[:, 2, :, mc * P:(mc + 1) * P],
                    h2[:, :, q * 512:(q + 1) * 512], start=True, stop=True,
                    perf_mode=DR)
        for mc in range(2):
            nc.vector.scalar_tensor_tensor(
                oT[:, mc, 0:w, :].rearrange("p a b -> p (a b)"),
                fT[mc][:, 0:nw], h8,
                x16[:, mc, 0:w, :].rearrange("p a b -> p (a b)"),
                ALU.mult, ALU.add)
        nc.sync.dma_start(oTd[:, :, t0 + 1:t0 + w + 1, :], oT[:, :, 0:w, :])
        return oT

    # chain state: carry slices; coll tiles batch 4 output frames per DMA
    ch = {"prev": None, "coll": None}

    def chain_step(k):
        """out[:,100+k+1] = y + h*f(y); y is the transposed f32 carry."""
        ytp = ch["prev"]
        y8 = y8p.tile([P, 2, P], FP8, tag="y8")
        nc.vector.tensor_copy(y8[:], ytp)
        c1 = chps.tile([P, 2, P], F32, tag="chp", name="c1")
        for mc in range(2):
            nc.tensor.matmul(c1[:, mc, :], bseed[:, :, 0, mc * P:(mc + 1) * P],
                             ones8[:], start=True, stop=False, perf_mode=DR)
            nc.tensor.matmul(c1[:, mc, :], w8[:, 0, :, mc * P:(mc + 1) * P],
                             y8[:], start=False, stop=True, perf_mode=DR)
        h1c = hcp.tile([P, 2, P], FP8, tag="h1c")
        nc.scalar.activation(h1c[:], c1[:], AF.Tanh, scale=0.125)
        c2 = chps.tile([P, 2, P], F32, tag="chp", name="c2")
        for mc in range(2):
            nc.tensor.matmul(c2[:, mc, :], bseed[:, :, 1, mc * P:(mc + 1) * P],
                             ones8[:], start=True, stop=False, perf_mode=DR)
            nc.tensor.matmul(c2[:, mc, :], w8[:, 1, :, mc * P:(mc + 1) * P],
                             h1c[:], start=False, stop=True, perf_mode=DR)
        h2c = hcp.tile([P, 2, P], FP8, tag="h2c")
        nc.scalar.activation(h2c[:], c2[:], AF.Tanh, scale=0.125)
        c3 = chps.tile([P, 2, P], F32, tag="chp", name="c3")
        for mc in range(2):
            nc.tensor.matmul(c3[:, mc, :], bseed[:, :, 2, mc * P:(mc + 1) * P],
                             ones8[:], start=True, stop=False, perf_mode=DR)
            nc.tensor.matmul(c3[:, mc, :], w8[:, 2, :, mc * P:(mc + 1) * P],
                             h2c[:], start=False, stop=True, perf_mode=DR)
        j = k % 4
        if j == 0:
            ch["coll"] = collp.tile([P, 2, 4, P], F32, tag="coll",
                                    name="coll")
        coll = ch["coll"]
        ytn = coll[:, :, j, :]
        nc.vector.scalar_tensor_tensor(ytn, c3[:], h8, ytp,
                                       ALU.mult, ALU.add)
        ch["prev"] = ytn
        if j == 3 or k == NSTEPS - 1:
            t0 = T_OBS + 1 + (k // 4) * 4
            nc.sync.dma_start(oTd[:, :, t0:t0 + j + 1, :], coll[:, :, 0:j + 1, :])

    # ---- schedule: last group first (chain dependency), then interleave ----
    oT_last = group(NG_FULL)              # frames 96..99 -> out 97..100
    ch["prev"] = oT_last[:, :, W_LAST - 1, :]   # y0 = out[:, 100]

    group(0)
    chain_step(0)
    step = 1
    for g in range(1, NG_FULL):
        group(g)
        for _ in range(2):
            if step < NSTEPS:
                chain_step(step)
                step += 1
    while step < NSTEPS:
        chain_step(step)
        step += 1


def _build(h):
    import concourse.mybir as mybir
    import concourse.tile as tile
    from concourse import bacc

    F32 = mybir.dt.float32
    FP16 = mybir.dt.float16
    FP8 = mybir.dt.float8e4

    nc = bacc.Bacc("TRN2", target_bir_lowering=False, debug=False,
                   num_devices=NCORES)
    xT8d = nc.dram_tensor("xT8", [P, 2, T_OBS, P], FP8,
                          kind="ExternalInput").ap()
    xT16d = nc.dram_tensor("xT16", [P, 2, T_OBS, P], FP16,
                           kind="ExternalInput").ap()
    w8d = nc.dram_tensor("w8", [P, 3, 2, D], FP8, kind="ExternalInput").ap()
    bactd = nc.dram_tensor("bact", [P, 4], F32, kind="ExternalInput").ap()
    bseedd = nc.dram_tensor("bseed", [1, 2, 3, D], FP8,
                            kind="ExternalInput").ap()
    ones8d = nc.dram_tensor("ones8", [1, 2, P], FP8, kind="ExternalInput").ap()
    oTd = nc.dram_tensor("oT", [P, 2, T, P], F32, kind="ExternalOutput").ap()

    with tile.TileContext(nc) as tc, ExitStack() as ctx:
        _emit(ctx, tc, xT8d, xT16d, w8d, bactd, bseedd, ones8d, oTd, h)
    nc.compile()
    return nc


def _host_inputs(inputs):
    """Shared (weights/bias) device arrays + h. Returns (h, shared)."""
    ts = np.asarray(inputs["time_steps"], np.float32)
    h = float(np.float32(ts[1]) - np.float32(ts[0]))

    f8 = ml_dtypes.float8_e4m3
    W1 = np.asarray(inputs["W1"], np.float32)
    W2 = np.asarray(inputs["W2"], np.float32)
    W3 = np.asarray(inputs["W3"], np.float32)
    b1 = np.asarray(inputs["b1"], np.float32)
    b2 = np.asarray(inputs["b2"], np.float32)
    b3 = np.asarray(inputs["b3"], np.float32)

    # w8[p, wi, kc, m] = 8*W_wi[kc*128+p, m]
    w8 = np.stack([8.0 * W1, 8.0 * W2, 8.0 * W3])  # [3, 256, 256]
    w8 = w8.reshape(3, 2, P, D).transpose(2, 0, 1, 3)  # [p, 3, kc, m]
    w8 = np.ascontiguousarray(w8).astype(f8)

    bact = np.stack([b1[:P], b1[P:], b2[:P], b2[P:]], axis=1)
    bact = np.ascontiguousarray(bact.astype(np.float32))

    # bseed[0, kc, wi, m] = 4*b_wi[m]  (K=2 DR ones contraction doubles it)
    bs = np.stack([4.0 * b1, 4.0 * b2, 4.0 * b3])  # [3, 256]
    bseed = np.broadcast_to(bs[None, None], (1, 2, 3, D))
    bseed = np.ascontiguousarray(bseed).astype(f8)
    ones8 = np.ones((1, 2, P), np.float32).astype(f8)

    shared = dict(w8=w8, bact=bact, bseed=bseed, ones8=ones8)
    return h, shared


def make_in_maps(inputs):
    """Full per-core input maps (shared + per-core transposed latents)."""
    h, shared = _host_inputs(inputs)
    b3 = np.asarray(inputs["b3"], np.float32)
    lat = np.ascontiguousarray(np.asarray(inputs["latents"], np.float32))
    f8 = ml_dtypes.float8_e4m3

    in_maps = []
    for c in range(NCORES):
        lc = lat[c * PB:(c + 1) * PB]                  # [128b, 100t, 256d]
        xt = lc.transpose(2, 1, 0)                     # [256d, 100t, 128b]
        xt8 = xt.reshape(2, P, T_OBS, P).transpose(1, 0, 2, 3)
        xt16 = (xt + (np.float32(h) * b3)[:, None, None])
        xt16 = xt16.reshape(2, P, T_OBS, P).transpose(1, 0, 2, 3)
        m = dict(shared)
        m["xT8"] = np.ascontiguousarray(xt8).astype(f8)
        m["xT16"] = np.ascontiguousarray(xt16).astype(np.float16)
        in_maps.append(m)
    return h, in_maps


def assemble_out(inputs, core_outs):
    """De-transpose per-core oT outputs and patch the exact copy frames."""
    lat = np.asarray(inputs["latents"], np.float32)
    out = np.empty((B, T, D), np.float32)
    for c in range(NCORES):
        oT = core_outs[c]                              # [128p, 2dc, 120t, 128b]
        out[c * PB:(c + 1) * PB] = oT.transpose(3, 2, 1, 0).reshape(PB, T, D)
    out[:, 0, :] = lat[:, 0, :]
    out[:, 2, :] = lat[:, 1, :]
    return out


_CACHE = {}


def kernel(**inputs):
    from concourse.bass_utils import run_bass_kernel_spmd

    h, in_maps = make_in_maps(inputs)
    if h not in _CACHE:
        _CACHE[h] = _build(h)
    nc = _CACHE[h]

    res = run_bass_kernel_spmd(nc, in_maps, list(range(NCORES)))
    outs = [res.results[c]["oT"] for c in range(NCORES)]
    return assemble_out(inputs, outs)


# revision 11
# speedup vs baseline: 1.2961x; 1.0182x over previous
"""Trainium2 Bass kernel for nn_ODE_71743133713072 (v2).

Semantics (unrolled from the reference lax.scan; time_steps = linspace, all
nonzero gaps equal h = ts[1]-ts[0]):
  out[:, 0]   = lat[:, 0]
  out[:, 2]   = lat[:, 1]                     (the scan's zero-length gap)
  out[:, t+1] = lat[:, t] + h * f(lat[:, t])  for t = 0..99, t != 1
  y = out[:, 100]
  out[:, k+1] = y = y + h * f(y)              for k = 100..118
where f is the D->U->U->D tanh MLP.

v2 layout strategy: everything on device lives FEATURE-ON-PARTITION
(transposed). The host pre-transposes the latents into
  xT8  [128p, 2dc, 100t, 128b] fp8   (matmul operand, unscaled)
  xT16 [128p, 2dc, 100t, 128b] fp16  (Euler base, with h*b3 pre-added)
and the device writes the output transposed (oT [128p, 2dc, 120t, 128b]
f32); the host de-transposes after gather. This removes every PE
transpose, every on-device cast, and every bias-seed matmul from the
parallel part: biases b1/b2 ride along as per-partition act biases, and
b3*h is folded into xT16. Matmuls are fp8 DoubleRow (K=256 in one pass,
0.5 cycles/col); weight loads pipeline behind the previous matmul.

The 19-step prediction chain keeps a transposed f32 carry, uses fp8-DR
for all three layers with tiny K=2 PSUM bias seeds, and is interleaved
between the parallel groups so its serial latency hides behind
DMA-bound group work.
"""

import os
import sys
from contextlib import ExitStack

import numpy as np

for _p in ("/opt/trn_rl_repo", "/root/.axon_site/_ro/trn_rl_repo"):
    if os.path.isdir(_p) and _p not in sys.path:
        sys.path.append(_p)

import ml_dtypes  # noqa: E402

B, T_OBS, KPRED, D = 1024, 100, 20, 256
T = T_OBS + KPRED          # 120
NCORES = 8
PB = B // NCORES           # 128 rows per core
P = 128
W = 8                      # frames per full group
NG_FULL = 12               # 12 full groups of 8 = 96 frames
W_LAST = 4                 # +1 group of 4 (frames 96..99)
NSTEPS = T - 1 - T_OBS     # 19 chain steps


def _emit(ctx, tc, xT8d, xT16d, w8d, bactd, bseedd, ones8d, oTd, h):
    import concourse.mybir as mybir

    nc = tc.nc
    F32 = mybir.dt.float32
    FP16 = mybir.dt.float16
    FP8 = mybir.dt.float8e4
    AF = mybir.ActivationFunctionType
    ALU = mybir.AluOpType
    DR = mybir.MatmulPerfMode.DoubleRow

    h8 = float(h / 8.0)

    const = ctx.enter_context(tc.tile_pool(name="const", bufs=1))
    w8 = const.tile([P, 3, 2, D], FP8, tag="w8")
    nc.sync.dma_start(w8[:], w8d[:])
    bact = const.tile([P, 4], F32, tag="bact")
    nc.sync.dma_start(bact[:], bactd[:])
    bseed = const.tile([1, 2, 3, D], FP8, tag="bseed")
    nc.sync.dma_start(bseed[:], bseedd[:])
    ones8 = const.tile([1, 2, P], FP8, tag="ones8")
    nc.sync.dma_start(ones8[:], ones8d[:])

    x16p = ctx.enter_context(tc.tile_pool(name="x16", bufs=3))
    x8p = ctx.enter_context(tc.tile_pool(name="x8", bufs=3))
    hp = ctx.enter_context(tc.tile_pool(name="hact", bufs=4))
    oTp = ctx.enter_context(tc.tile_pool(name="oT", bufs=3))
    psp = ctx.enter_context(tc.tile_pool(name="ps", bufs=3, space="PSUM"))

    y8p = ctx.enter_context(tc.tile_pool(name="y8", bufs=2))
    hcp = ctx.enter_context(tc.tile_pool(name="hc", bufs=4))
    collp = ctx.enter_context(tc.tile_pool(name="coll", bufs=2))
    chps = ctx.enter_context(tc.tile_pool(name="chps", bufs=2, space="PSUM"))

    def g_load(g):
        """DMA-in for group g; returns (x16, x8)."""
        w = W if g < NG_FULL else W_LAST
        t0 = g * W
        x16 = x16p.tile([P, 2, W, P], FP16, tag="x16")
        nc.sync.dma_start(x16[:, :, 0:w, :], xT16d[:, :, t0:t0 + w, :])
        x8 = x8p.tile([P, 2, W, P], FP8, tag="x8")
        nc.sync.dma_start(x8[:, :, 0:w, :], xT8d[:, :, t0:t0 + w, :])
        return x16, x8

    def g_layer(g, wi, rhs_of, out_fp8):
        """One MLP layer for group g: 2mc x w/4 matmuls + per-mc tanh."""
        w = W if g < NG_FULL else W_LAST
        nw = w * P
        mm = [psp.tile([P, W * P], F32, tag="ps", name="mm") for _ in range(2)]
        for mc in range(2):
            for q in range(w // 4):
                nc.tensor.matmul(
                    mm[mc][:, q * 512:(q + 1) * 512],
                    w8[:, wi, :, mc * P:(mc + 1) * P],
                    rhs_of(q), start=True, stop=True, perf_mode=DR)
        if out_fp8 is not None:
            for mc in range(2):
                nc.scalar.activation(out_fp8[:, mc, 0:nw], mm[mc][:, 0:nw],
                                     AF.Tanh, bias=bact[:, 2 * wi + mc:
                                                        2 * wi + mc + 1],
                                     scale=0.125)
        return mm

    def g_store(g, fT, x16):
        """Euler update + DMA-out for group g."""
        w = W if g < NG_FULL else W_LAST
        t0 = g * W
        nw = w * P
        oT = oTp.tile([P, 2, W, P], F32, tag="oT")
        for mc in range(2):
            nc.vector.scalar_tensor_tensor(
                oT[:, mc, 0:w, :].rearrange("p a b -> p (a b)"),
                fT[mc][:, 0:nw], h8,
                x16[:, mc, 0:w, :].rearrange("p a b -> p (a b)"),
                ALU.mult, ALU.add)
        nc.sync.dma_start(oTd[:, :, t0 + 1:t0 + w + 1, :], oT[:, :, 0:w, :])
        return oT

    def group(g, mid1=None, mid2=None):
        """Full group with optional fill-work closures at PE stall points."""
        x16, x8 = g_load(g)
        h1 = hp.tile([P, 2, W * P], FP8, tag="h1")
        h2 = hp.tile([P, 2, W * P], FP8, tag="h2")
        g_layer(g, 0, lambda q: x8[:, :, 4 * q:4 * q + 4, :], h1)
        if mid1 is not None:
            mid1()
        g_layer(g, 1, lambda q: h1[:, :, q * 512:(q + 1) * 512], h2)
        if mid2 is not None:
            mid2()
        fT = g_layer(g, 2, lambda q: h2[:, :, q * 512:(q + 1) * 512], None)
        return g_store(g, fT, x16)

    # chain state: carry slices; coll tiles batch 4 output frames per DMA
    ch = {"prev": None, "coll": None}

    def chain_step(k):
        """out[:,100+k+1] = y + h*f(y); y is the transposed f32 carry."""
        ytp = ch["prev"]
        y8 = y8p.tile([P, 2, P], FP8, tag="y8")
        nc.vector.tensor_copy(y8[:], ytp)
        c1 = chps.tile([P, 2, P], F32, tag="chp", name="c1")
        for mc in range(2):
            nc.tensor.matmul(c1[:, mc, :], bseed[:, :, 0, mc * P:(mc + 1) * P],
                             ones8[:], start=True, stop=False, perf_mode=DR)
            nc.tensor.matmul(c1[:, mc, :], w8[:, 0, :, mc * P:(mc + 1) * P],
                             y8[:], start=False, stop=True, perf_mode=DR)
        h1c = hcp.tile([P, 2, P], FP8, tag="h1c")
        nc.scalar.activation(h1c[:], c1[:], AF.Tanh, scale=0.125)
        c2 = chps.tile([P, 2, P], F32, tag="chp", name="c2")
        for mc in range(2):
            nc.tensor.matmul(c2[:, mc, :], bseed[:, :, 1, mc * P:(mc + 1) * P],
                             ones8[:], start=True, stop=False, perf_mode=DR)
            nc.tensor.matmul(c2[:, mc, :], w8[:, 1, :, mc * P:(mc + 1) * P],
                             h1c[:], start=False, stop=True, perf_mode=DR)
        h2c = hcp.tile([P, 2, P], FP8, tag="h2c")
        nc.scalar.activation(h2c[:], c2[:], AF.Tanh, scale=0.125)
        c3 = chps.tile([P, 2, P], F32, tag="chp", name="c3")
        for mc in range(2):
            nc.tensor.matmul(c3[:, mc, :], bseed[:, :, 2, mc * P:(mc + 1) * P],
                             ones8[:], start=True, stop=False, perf_mode=DR)
            nc.tensor.matmul(c3[:, mc, :], w8[:, 2, :, mc * P:(mc + 1) * P],
                             h2c[:], start=False, stop=True, perf_mode=DR)
        j = k % 4
        if j == 0:
            ch["coll"] = collp.tile([P, 2, 4, P], F32, tag="coll",
                                    name="coll")
        coll = ch["coll"]
        ytn = coll[:, :, j, :]
        nc.vector.scalar_tensor_tensor(ytn, c3[:], h8, ytp,
                                       ALU.mult, ALU.add)
        ch["prev"] = ytn
        if j == 3 or k == NSTEPS - 1:
            t0 = T_OBS + 1 + (k // 4) * 4
            nc.sync.dma_start(oTd[:, :, t0:t0 + j + 1, :], coll[:, :, 0:j + 1, :])

    # ---- schedule: last group first (chain dependency), then the rest with
    # chain steps embedded at the PE stall points (post-L1/post-L2, where the
    # tensor queue would otherwise wait on the Act engine) ----
    oT_last = group(NG_FULL)              # frames 96..99 -> out 97..100
    ch["prev"] = oT_last[:, :, W_LAST - 1, :]   # y0 = out[:, 100]

    state = {"k": 0}

    def fill():
        if state["k"] < NSTEPS:
            chain_step(state["k"])
            state["k"] += 1

    for g in range(NG_FULL):
        group(g, mid1=fill, mid2=fill)
    while state["k"] < NSTEPS:
        fill()


def _build(h):
    import concourse.mybir as mybir
    import concourse.tile as tile
    from concourse import bacc

    F32 = mybir.dt.float32
    FP16 = mybir.dt.float16
    FP8 = mybir.dt.float8e4

    nc = bacc.Bacc("TRN2", target_bir_lowering=False, debug=False,
                   num_devices=NCORES)
    xT8d = nc.dram_tensor("xT8", [P, 2, T_OBS, P], FP8,
                          kind="ExternalInput").ap()
    xT16d = nc.dram_tensor("xT16", [P, 2, T_OBS, P], FP16,
                           kind="ExternalInput").ap()
    w8d = nc.dram_tensor("w8", [P, 3, 2, D], FP8, kind="ExternalInput").ap()
    bactd = nc.dram_tensor("bact", [P, 4], F32, kind="ExternalInput").ap()
    bseedd = nc.dram_tensor("bseed", [1, 2, 3, D], FP8,
                            kind="ExternalInput").ap()
    ones8d = nc.dram_tensor("ones8", [1, 2, P], FP8, kind="ExternalInput").ap()
    oTd = nc.dram_tensor("oT", [P, 2, T, P], F32, kind="ExternalOutput").ap()

    with tile.TileContext(nc) as tc, ExitStack() as ctx:
        _emit(ctx, tc, xT8d, xT16d, w8d, bactd, bseedd, ones8d, oTd, h)
    nc.compile()
    return nc


def _host_inputs(inputs):
    """Shared (weights/bias) device arrays + h. Returns (h, shared)."""
    ts = np.asarray(inputs["time_steps"], np.float32)
    h = float(np.float32(ts[1]) - np.float32(ts[0]))

    f8 = ml_dtypes.float8_e4m3
    W1 = np.asarray(inputs["W1"], np.float32)
    W2 = np.asarray(inputs["W2"], np.float32)
    W3 = np.asarray(inputs["W3"], np.float32)
    b1 = np.asarray(inputs["b1"], np.float32)
    b2 = np.asarray(inputs["b2"], np.float32)
    b3 = np.asarray(inputs["b3"], np.float32)

    # w8[p, wi, kc, m] = 8*W_wi[kc*128+p, m]
    w8 = np.stack([8.0 * W1, 8.0 * W2, 8.0 * W3])  # [3, 256, 256]
    w8 = w8.reshape(3, 2, P, D).transpose(2, 0, 1, 3)  # [p, 3, kc, m]
    w8 = np.ascontiguousarray(w8).astype(f8)

    bact = np.stack([b1[:P], b1[P:], b2[:P], b2[P:]], axis=1)
    bact = np.ascontiguousarray(bact.astype(np.float32))

    # bseed[0, kc, wi, m] = 4*b_wi[m]  (K=2 DR ones contraction doubles it)
    bs = np.stack([4.0 * b1, 4.0 * b2, 4.0 * b3])  # [3, 256]
    bseed = np.broadcast_to(bs[None, None], (1, 2, 3, D))
    bseed = np.ascontiguousarray(bseed).astype(f8)
    ones8 = np.ones((1, 2, P), np.float32).astype(f8)

    shared = dict(w8=w8, bact=bact, bseed=bseed, ones8=ones8)
    return h, shared


def make_in_maps(inputs):
    """Full per-core input maps (shared + per-core transposed latents)."""
    h, shared = _host_inputs(inputs)
    b3 = np.asarray(inputs["b3"], np.float32)
    lat = np.ascontiguousarray(np.asarray(inputs["latents"], np.float32))
    f8 = ml_dtypes.float8_e4m3

    in_maps = []
    for c in range(NCORES):
        lc = lat[c * PB:(c + 1) * PB]                  # [128b, 100t, 256d]
        xt = lc.transpose(2, 1, 0)                     # [256d, 100t, 128b]
        xt8 = xt.reshape(2, P, T_OBS, P).transpose(1, 0, 2, 3)
        xt16 = (xt + (np.float32(h) * b3)[:, None, None])
        xt16 = xt16.reshape(2, P, T_OBS, P).transpose(1, 0, 2, 3)
        m = dict(shared)
        m["xT8"] = np.ascontiguousarray(xt8).astype(f8)
        m["xT16"] = np.ascontiguousarray(xt16).astype(np.float16)
        in_maps.append(m)
    return h, in_maps


def assemble_out(inputs, core_outs):
    """De-transpose per-core oT outputs and patch the exact copy frames."""
    lat = np.asarray(inputs["latents"], np.float32)
    out = np.empty((B, T, D), np.float32)
    for c in range(NCORES):
        oT = core_outs[c]                              # [128p, 2dc, 120t, 128b]
        out[c * PB:(c + 1) * PB] = oT.transpose(3, 2, 1, 0).reshape(PB, T, D)
    out[:, 0, :] = lat[:, 0, :]
    out[:, 2, :] = lat[:, 1, :]
    return out


_CACHE = {}


def kernel(**inputs):
    from concourse.bass_utils import run_bass_kernel_spmd

    h, in_maps = make_in_maps(inputs)
    if h not in _CACHE:
        _CACHE[h] = _build(h)
    nc = _CACHE[h]

    res = run_bass_kernel_spmd(nc, in_maps, list(range(NCORES)))
    outs = [res.results[c]["oT"] for c in range(NCORES)]
    return assemble_out(inputs, outs)


# revision 19
# speedup vs baseline: 1.6759x; 1.2930x over previous
"""Trainium2 Bass kernel for nn_ODE_71743133713072 (v2).

Semantics (unrolled from the reference lax.scan; time_steps = linspace, all
nonzero gaps equal h = ts[1]-ts[0]):
  out[:, 0]   = lat[:, 0]
  out[:, 2]   = lat[:, 1]                     (the scan's zero-length gap)
  out[:, t+1] = lat[:, t] + h * f(lat[:, t])  for t = 0..99, t != 1
  y = out[:, 100]
  out[:, k+1] = y = y + h * f(y)              for k = 100..118
where f is the D->U->U->D tanh MLP.

v2 layout strategy: everything on device lives FEATURE-ON-PARTITION
(transposed). The host pre-transposes the latents into
  xT8  [128p, 2dc, 100t, 128b] fp8   (matmul operand, unscaled)
  xT16 [128p, 2dc, 100t, 128b] fp16  (Euler base, with h*b3 pre-added)
and the device writes the output transposed (oT [128p, 2dc, 120t, 128b]
f32); the host de-transposes after gather. This removes every PE
transpose, every on-device cast, and every bias-seed matmul from the
parallel part: biases b1/b2 ride along as per-partition act biases, and
b3*h is folded into xT16. Matmuls are fp8 DoubleRow (K=256 in one pass,
0.5 cycles/col); weight loads pipeline behind the previous matmul.

The 19-step prediction chain keeps a transposed f32 carry, uses fp8-DR
for all three layers with tiny K=2 PSUM bias seeds, and is interleaved
between the parallel groups so its serial latency hides behind
DMA-bound group work.
"""

import os
import sys
from contextlib import ExitStack

import numpy as np

for _p in ("/opt/trn_rl_repo", "/root/.axon_site/_ro/trn_rl_repo"):
    if os.path.isdir(_p) and _p not in sys.path:
        sys.path.append(_p)

import ml_dtypes  # noqa: E402

B, T_OBS, KPRED, D = 1024, 100, 20, 256
T = T_OBS + KPRED          # 120
NCORES = 8
PB = B // NCORES           # 128 rows per core
P = 128
W = 8                      # frames per full group
NG_FULL = 12               # 12 full groups of 8 = 96 frames
W_LAST = 4                 # +1 group of 4 (frames 96..99)
NSTEPS = T - 1 - T_OBS     # 19 chain steps


def _emit(ctx, tc, xT8d, xT16d, w8d, bactd, bseedd, ones8d, w16d, b16d,
          oTd, h):
    import concourse.mybir as mybir

    nc = tc.nc
    F32 = mybir.dt.float32
    FP16 = mybir.dt.float16
    FP8 = mybir.dt.float8e4
    AF = mybir.ActivationFunctionType
    ALU = mybir.AluOpType
    DR = mybir.MatmulPerfMode.DoubleRow

    h8 = float(h / 8.0)

    const = ctx.enter_context(tc.tile_pool(name="const", bufs=1))
    w8 = const.tile([P, 3, 2, D], FP8, tag="w8")
    nc.sync.dma_start(w8[:], w8d[:])
    bact = const.tile([P, 4], F32, tag="bact")
    nc.sync.dma_start(bact[:], bactd[:])
    bseed = const.tile([1, 2, 3, D], FP8, tag="bseed")
    nc.sync.dma_start(bseed[:], bseedd[:])
    ones8 = const.tile([1, 2, P], FP8, tag="ones8")
    nc.sync.dma_start(ones8[:], ones8d[:])
    w16 = const.tile([P, 2, D], FP16, tag="w16")
    nc.sync.dma_start(w16[:], w16d[:])
    b16 = const.tile([1, D + P], FP16, tag="b16")  # [b1 (256) | ones (128)]
    nc.sync.dma_start(b16[:], b16d[:])

    x16p = ctx.enter_context(tc.tile_pool(name="x16", bufs=3))
    x8p = ctx.enter_context(tc.tile_pool(name="x8", bufs=3))
    hp = ctx.enter_context(tc.tile_pool(name="hact", bufs=4))
    oTp = ctx.enter_context(tc.tile_pool(name="oT", bufs=3))
    psp = ctx.enter_context(tc.tile_pool(name="ps", bufs=3, space="PSUM"))

    y16p = ctx.enter_context(tc.tile_pool(name="y16", bufs=2))
    hcp = ctx.enter_context(tc.tile_pool(name="hc", bufs=4))
    collp = ctx.enter_context(tc.tile_pool(name="coll", bufs=2))
    chps = ctx.enter_context(tc.tile_pool(name="chps", bufs=2, space="PSUM"))

    def g_load(g):
        """DMA-in for group g; returns (x16, x8)."""
        w = W if g < NG_FULL else W_LAST
        t0 = g * W
        x16 = x16p.tile([P, 2, W, P], FP16, tag="x16")
        nc.sync.dma_start(x16[:, :, 0:w, :], xT16d[:, :, t0:t0 + w, :])
        x8 = x8p.tile([P, 2, W, P], FP8, tag="x8")
        nc.sync.dma_start(x8[:, :, 0:w, :], xT8d[:, :, t0:t0 + w, :])
        return x16, x8

    def g_layer(g, wi, rhs_of, out_fp8):
        """One MLP layer for group g: 2mc x w/4 matmuls + per-mc tanh."""
        w = W if g < NG_FULL else W_LAST
        nw = w * P
        mm = [psp.tile([P, W * P], F32, tag="ps", name="mm") for _ in range(2)]
        for mc in range(2):
            for q in range(w // 4):
                nc.tensor.matmul(
                    mm[mc][:, q * 512:(q + 1) * 512],
                    w8[:, wi, :, mc * P:(mc + 1) * P],
                    rhs_of(q), start=True, stop=True, perf_mode=DR)
        if out_fp8 is not None:
            for mc in range(2):
                nc.scalar.activation(out_fp8[:, mc, 0:nw], mm[mc][:, 0:nw],
                                     AF.Tanh, bias=bact[:, 2 * wi + mc:
                                                        2 * wi + mc + 1],
                                     scale=0.125)
        return mm

    def g_store(g, fT, x16):
        """Euler update + DMA-out for group g."""
        w = W if g < NG_FULL else W_LAST
        t0 = g * W
        nw = w * P
        oT = oTp.tile([P, 2, W, P], F32, tag="oT")
        for mc in range(2):
            nc.vector.scalar_tensor_tensor(
                oT[:, mc, 0:w, :].rearrange("p a b -> p (a b)"),
                fT[mc][:, 0:nw], h8,
                x16[:, mc, 0:w, :].rearrange("p a b -> p (a b)"),
                ALU.mult, ALU.add)
        nc.sync.dma_start(oTd[:, :, t0 + 1:t0 + w + 1, :], oT[:, :, 0:w, :])
        return oT

    loads = {}

    def group(g, mid1=None, mid2=None, prefetch=None):
        """Full group with optional fill-work closures at PE stall points."""
        x16, x8 = loads.pop(g, None) or g_load(g)
        h1 = hp.tile([P, 2, W * P], FP8, tag="h1")
        h2 = hp.tile([P, 2, W * P], FP8, tag="h2")
        g_layer(g, 0, lambda q: x8[:, :, 4 * q:4 * q + 4, :], h1)
        if prefetch is not None and prefetch not in loads:
            loads[prefetch] = g_load(prefetch)
        if mid1 is not None:
            mid1()
        g_layer(g, 1, lambda q: h1[:, :, q * 512:(q + 1) * 512], h2)
        if mid2 is not None:
            mid2()
        fT = g_layer(g, 2, lambda q: h2[:, :, q * 512:(q + 1) * 512], None)
        return g_store(g, fT, x16)

    # chain state: carry slices; coll tiles batch 4 output frames per DMA
    ch = {"prev": None, "coll": None}

    def chain_step(k):
        """out[:,100+k+1] = y + h*f(y); y is the transposed fp16 carry."""
        ytp = ch["prev"]   # [P, 2, P] fp16
        # L1 in fp16 straight off the carry (no cast hop): exact b1 seed
        # (K=1) + two kc-accumulated fp16 matmuls per mc chunk.
        c1 = chps.tile([P, 2, P], F32, tag="chp", name="c1")
        for mc in range(2):
            nc.tensor.matmul(c1[:, mc, :], b16[:, mc * P:(mc + 1) * P],
                             b16[:, D:D + P], start=True, stop=False)
        for mc in range(2):
            for kc in range(2):
                nc.tensor.matmul(c1[:, mc, :],
                                 w16[:, kc, mc * P:(mc + 1) * P],
                                 ytp[:, kc, :], start=False, stop=(kc == 1))
        h1c = hcp.tile([P, 2, P], FP8, tag="h1c")
        nc.scalar.activation(h1c[:], c1[:], AF.Tanh)
        c2 = chps.tile([P, 2, P], F32, tag="chp", name="c2")
        for mc in range(2):
            nc.tensor.matmul(c2[:, mc, :], bseed[:, :, 1, mc * P:(mc + 1) * P],
                             ones8[:], start=True, stop=False, perf_mode=DR)
        for mc in range(2):
            nc.tensor.matmul(c2[:, mc, :], w8[:, 1, :, mc * P:(mc + 1) * P],
                             h1c[:], start=False, stop=True, perf_mode=DR)
        h2c = hcp.tile([P, 2, P], FP8, tag="h2c")
        nc.scalar.activation(h2c[:], c2[:], AF.Tanh, scale=0.125)
        c3 = chps.tile([P, 2, P], F32, tag="chp", name="c3")
        for mc in range(2):
            nc.tensor.matmul(c3[:, mc, :], bseed[:, :, 2, mc * P:(mc + 1) * P],
                             ones8[:], start=True, stop=False, perf_mode=DR)
        for mc in range(2):
            nc.tensor.matmul(c3[:, mc, :], w8[:, 2, :, mc * P:(mc + 1) * P],
                             h2c[:], start=False, stop=True, perf_mode=DR)
        j = k % 4
        if j == 0:
            ch["coll"] = collp.tile([P, 2, 4, P], F32, tag="coll",
                                    name="coll")
        coll = ch["coll"]
        ytn = y16p.tile([P, 2, P], FP16, tag="ytn")
        nc.vector.scalar_tensor_tensor(ytn[:], c3[:], h8, ytp,
                                       ALU.mult, ALU.add)
        ch["prev"] = ytn
        nc.vector.scalar_tensor_tensor(coll[:, :, j, :], c3[:], h8, ytp,
                                       ALU.mult, ALU.add)
        if j == 3 or k == NSTEPS - 1:
            t0 = T_OBS + 1 + (k // 4) * 4
            nc.sync.dma_start(oTd[:, :, t0:t0 + j + 1, :], coll[:, :, 0:j + 1, :])

    # ---- schedule: PE warmup, last group first (chain dependency), then the
    # rest with chain steps embedded at the PE stall points (post-L1/post-L2,
    # where the tensor queue would otherwise wait on the Act engine) ----
    warm = chps.tile([P, 2, P], F32, tag="chp", name="warm")
    for i in range(10):
        nc.tensor.matmul(warm[:, i % 2, :], bseed[:, :, 0, 0:P], ones8[:],
                         start=True, stop=True, perf_mode=DR)
    wsink = hcp.tile([P, 2, P], FP8, tag="wsink")
    nc.scalar.activation(wsink[:], warm[:], AF.Tanh)

    oT_last = group(NG_FULL, prefetch=0)  # frames 96..99 -> out 97..100
    y0 = y16p.tile([P, 2, P], FP16, tag="y0")
    nc.vector.tensor_copy(y0[:], oT_last[:, :, W_LAST - 1, :])
    ch["prev"] = y0                       # y0 = out[:, 100] as fp16

    state = {"k": 0}

    def fill():
        if state["k"] < NSTEPS:
            chain_step(state["k"])
            state["k"] += 1

    for g in range(NG_FULL):
        group(g, mid1=fill, mid2=fill,
              prefetch=g + 1 if g + 1 < NG_FULL else None)
    while state["k"] < NSTEPS:
        fill()


def _build(h):
    import concourse.mybir as mybir
    import concourse.tile as tile
    from concourse import bacc

    F32 = mybir.dt.float32
    FP16 = mybir.dt.float16
    FP8 = mybir.dt.float8e4

    nc = bacc.Bacc("TRN2", target_bir_lowering=False, debug=False,
                   num_devices=NCORES)
    xT8d = nc.dram_tensor("xT8", [P, 2, T_OBS, P], FP8,
                          kind="ExternalInput").ap()
    xT16d = nc.dram_tensor("xT16", [P, 2, T_OBS, P], FP16,
                           kind="ExternalInput").ap()
    w8d = nc.dram_tensor("w8", [P, 3, 2, D], FP8, kind="ExternalInput").ap()
    bactd = nc.dram_tensor("bact", [P, 4], F32, kind="ExternalInput").ap()
    bseedd = nc.dram_tensor("bseed", [1, 2, 3, D], FP8,
                            kind="ExternalInput").ap()
    ones8d = nc.dram_tensor("ones8", [1, 2, P], FP8, kind="ExternalInput").ap()
    w16d = nc.dram_tensor("w16", [P, 2, D], FP16, kind="ExternalInput").ap()
    b16d = nc.dram_tensor("b16", [1, D + P], FP16, kind="ExternalInput").ap()
    oTd = nc.dram_tensor("oT", [P, 2, T, P], F32, kind="ExternalOutput").ap()

    with tile.TileContext(nc) as tc, ExitStack() as ctx:
        _emit(ctx, tc, xT8d, xT16d, w8d, bactd, bseedd, ones8d, w16d, b16d,
              oTd, h)
    nc.compile()
    return nc


def _host_inputs(inputs):
    """Shared (weights/bias) device arrays + h. Returns (h, shared)."""
    ts = np.asarray(inputs["time_steps"], np.float32)
    h = float(np.float32(ts[1]) - np.float32(ts[0]))

    f8 = ml_dtypes.float8_e4m3
    W1 = np.asarray(inputs["W1"], np.float32)
    W2 = np.asarray(inputs["W2"], np.float32)
    W3 = np.asarray(inputs["W3"], np.float32)
    b1 = np.asarray(inputs["b1"], np.float32)
    b2 = np.asarray(inputs["b2"], np.float32)
    b3 = np.asarray(inputs["b3"], np.float32)

    # w8[p, wi, kc, m] = 8*W_wi[kc*128+p, m]
    w8 = np.stack([8.0 * W1, 8.0 * W2, 8.0 * W3])  # [3, 256, 256]
    w8 = w8.reshape(3, 2, P, D).transpose(2, 0, 1, 3)  # [p, 3, kc, m]
    w8 = np.ascontiguousarray(w8).astype(f8)

    bact = np.stack([b1[:P], b1[P:], b2[:P], b2[P:]], axis=1)
    bact = np.ascontiguousarray(bact.astype(np.float32))

    # bseed[0, kc, wi, m] = 4*b_wi[m]  (K=2 DR ones contraction doubles it)
    bs = np.stack([4.0 * b1, 4.0 * b2, 4.0 * b3])  # [3, 256]
    bseed = np.broadcast_to(bs[None, None], (1, 2, 3, D))
    bseed = np.ascontiguousarray(bseed).astype(f8)
    ones8 = np.ones((1, 2, P), np.float32).astype(f8)

    # chain L1 runs in fp16: exact W1/b1 (unscaled) + a ones row
    w16 = W1.reshape(2, P, D).transpose(1, 0, 2)   # [p, kc, m]
    w16 = np.ascontiguousarray(w16).astype(np.float16)
    b16 = np.concatenate([b1, np.ones(P, np.float32)]).reshape(1, D + P)
    b16 = b16.astype(np.float16)

    shared = dict(w8=w8, bact=bact, bseed=bseed, ones8=ones8,
                  w16=w16, b16=b16)
    return h, shared


def make_in_maps(inputs):
    """Full per-core input maps (shared + per-core transposed latents)."""
    h, shared = _host_inputs(inputs)
    b3 = np.asarray(inputs["b3"], np.float32)
    lat = np.ascontiguousarray(np.asarray(inputs["latents"], np.float32))
    f8 = ml_dtypes.float8_e4m3

    in_maps = []
    for c in range(NCORES):
        lc = lat[c * PB:(c + 1) * PB]                  # [128b, 100t, 256d]
        xt = lc.transpose(2, 1, 0)                     # [256d, 100t, 128b]
        xt8 = xt.reshape(2, P, T_OBS, P).transpose(1, 0, 2, 3)
        xt16 = (xt + (np.float32(h) * b3)[:, None, None])
        xt16 = xt16.reshape(2, P, T_OBS, P).transpose(1, 0, 2, 3)
        m = dict(shared)
        m["xT8"] = np.ascontiguousarray(xt8).astype(f8)
        m["xT16"] = np.ascontiguousarray(xt16).astype(np.float16)
        in_maps.append(m)
    return h, in_maps


def assemble_out(inputs, core_outs):
    """De-transpose per-core oT outputs and patch the exact copy frames."""
    lat = np.asarray(inputs["latents"], np.float32)
    out = np.empty((B, T, D), np.float32)
    for c in range(NCORES):
        oT = core_outs[c]                              # [128p, 2dc, 120t, 128b]
        out[c * PB:(c + 1) * PB] = oT.transpose(3, 2, 1, 0).reshape(PB, T, D)
    out[:, 0, :] = lat[:, 0, :]
    out[:, 2, :] = lat[:, 1, :]
    return out


_CACHE = {}


def kernel(**inputs):
    from concourse.bass_utils import run_bass_kernel_spmd

    h, in_maps = make_in_maps(inputs)
    if h not in _CACHE:
        _CACHE[h] = _build(h)
    nc = _CACHE[h]

    res = run_bass_kernel_spmd(nc, in_maps, list(range(NCORES)))
    outs = [res.results[c]["oT"] for c in range(NCORES)]
    return assemble_out(inputs, outs)


# revision 21
# speedup vs baseline: 1.9922x; 1.1888x over previous
"""Trainium2 Bass kernel for nn_ODE_71743133713072 (v2).

Semantics (unrolled from the reference lax.scan; time_steps = linspace, all
nonzero gaps equal h = ts[1]-ts[0]):
  out[:, 0]   = lat[:, 0]
  out[:, 2]   = lat[:, 1]                     (the scan's zero-length gap)
  out[:, t+1] = lat[:, t] + h * f(lat[:, t])  for t = 0..99, t != 1
  y = out[:, 100]
  out[:, k+1] = y = y + h * f(y)              for k = 100..118
where f is the D->U->U->D tanh MLP.

v2 layout strategy: everything on device lives FEATURE-ON-PARTITION
(transposed). The host pre-transposes the latents into
  xT8  [128p, 2dc, 100t, 128b] fp8   (matmul operand, unscaled)
  xT16 [128p, 2dc, 100t, 128b] fp16  (Euler base, with h*b3 pre-added)
and the device writes the output transposed (oT [128p, 2dc, 120t, 128b]
f32); the host de-transposes after gather. This removes every PE
transpose, every on-device cast, and every bias-seed matmul from the
parallel part: biases b1/b2 ride along as per-partition act biases, and
b3*h is folded into xT16. Matmuls are fp8 DoubleRow (K=256 in one pass,
0.5 cycles/col); weight loads pipeline behind the previous matmul.

The 19-step prediction chain keeps a transposed f32 carry, uses fp8-DR
for all three layers with tiny K=2 PSUM bias seeds, and is interleaved
between the parallel groups so its serial latency hides behind
DMA-bound group work.
"""

import os
import sys
from contextlib import ExitStack

import numpy as np

for _p in ("/opt/trn_rl_repo", "/root/.axon_site/_ro/trn_rl_repo"):
    if os.path.isdir(_p) and _p not in sys.path:
        sys.path.append(_p)

import ml_dtypes  # noqa: E402

B, T_OBS, KPRED, D = 1024, 100, 20, 256
T = T_OBS + KPRED          # 120
NCORES = 8
PB = B // NCORES           # 128 rows per core
P = 128
W = 8                      # frames per full group
NG_FULL = 12               # 12 full groups of 8 = 96 frames
W_LAST = 4                 # +1 group of 4 (frames 96..99)
NSTEPS = T - 1 - T_OBS     # 19 chain steps


def _emit(ctx, tc, xT8d, xT16d, w8d, bactd, bseedd, ones8d, w16d, b16d,
          oTd, h):
    import concourse.mybir as mybir

    nc = tc.nc
    F32 = mybir.dt.float32
    FP16 = mybir.dt.float16
    FP8 = mybir.dt.float8e4
    AF = mybir.ActivationFunctionType
    ALU = mybir.AluOpType
    DR = mybir.MatmulPerfMode.DoubleRow

    h8 = float(h / 8.0)

    const = ctx.enter_context(tc.tile_pool(name="const", bufs=1))
    w8 = const.tile([P, 3, 2, D], FP8, tag="w8")
    nc.sync.dma_start(w8[:], w8d[:])
    bact = const.tile([P, 4], F32, tag="bact")
    nc.sync.dma_start(bact[:], bactd[:])
    bseed = const.tile([1, 2, 3, D], FP8, tag="bseed")
    nc.sync.dma_start(bseed[:], bseedd[:])
    ones8 = const.tile([1, 2, P], FP8, tag="ones8")
    nc.sync.dma_start(ones8[:], ones8d[:])
    w16 = const.tile([P, 2, D], FP16, tag="w16")
    nc.sync.dma_start(w16[:], w16d[:])
    b16 = const.tile([1, D + P], FP16, tag="b16")  # [b1 (256) | ones (128)]
    nc.sync.dma_start(b16[:], b16d[:])

    x16p = ctx.enter_context(tc.tile_pool(name="x16", bufs=3))
    x8p = ctx.enter_context(tc.tile_pool(name="x8", bufs=3))
    hp = ctx.enter_context(tc.tile_pool(name="hact", bufs=4))
    oTp = ctx.enter_context(tc.tile_pool(name="oT", bufs=3))
    psp = ctx.enter_context(tc.tile_pool(name="ps", bufs=3, space="PSUM"))

    y16p = ctx.enter_context(tc.tile_pool(name="y16", bufs=2))
    hcp = ctx.enter_context(tc.tile_pool(name="hc", bufs=4))
    collp = ctx.enter_context(tc.tile_pool(name="coll", bufs=2))
    chps = ctx.enter_context(tc.tile_pool(name="chps", bufs=2, space="PSUM"))

    def g_load(g):
        """DMA-in for group g; returns (x16, x8)."""
        w = W if g < NG_FULL else W_LAST
        t0 = g * W
        x16 = x16p.tile([P, 2, W, P], FP16, tag="x16")
        nc.sync.dma_start(x16[:, :, 0:w, :], xT16d[:, :, t0:t0 + w, :])
        x8 = x8p.tile([P, 2, W, P], FP8, tag="x8")
        nc.sync.dma_start(x8[:, :, 0:w, :], xT8d[:, :, t0:t0 + w, :])
        return x16, x8

    def g_layer(g, wi, rhs_of, out_fp8):
        """One MLP layer for group g: 2mc x w/4 matmuls + per-mc tanh."""
        w = W if g < NG_FULL else W_LAST
        nw = w * P
        mm = [psp.tile([P, W * P], F32, tag="ps", name="mm") for _ in range(2)]
        for mc in range(2):
            for q in range(w // 4):
                nc.tensor.matmul(
                    mm[mc][:, q * 512:(q + 1) * 512],
                    w8[:, wi, :, mc * P:(mc + 1) * P],
                    rhs_of(q), start=True, stop=True, perf_mode=DR)
        if out_fp8 is not None:
            for mc in range(2):
                nc.scalar.activation(out_fp8[:, mc, 0:nw], mm[mc][:, 0:nw],
                                     AF.Tanh, bias=bact[:, 2 * wi + mc:
                                                        2 * wi + mc + 1],
                                     scale=0.125)
        return mm

    def g_store(g, fT, x16):
        """Euler update + DMA-out for group g."""
        w = W if g < NG_FULL else W_LAST
        t0 = g * W
        nw = w * P
        oT = oTp.tile([P, 2, W, P], F32, tag="oT")
        for mc in range(2):
            nc.vector.scalar_tensor_tensor(
                oT[:, mc, 0:w, :].rearrange("p a b -> p (a b)"),
                fT[mc][:, 0:nw], h8,
                x16[:, mc, 0:w, :].rearrange("p a b -> p (a b)"),
                ALU.mult, ALU.add)
        nc.sync.dma_start(oTd[:, :, t0 + 1:t0 + w + 1, :], oT[:, :, 0:w, :])
        return oT

    loads = {}

    def group(g, mid1=None, mid2=None, prefetch=None, pend=None):
        """Group slot. With `pend`, L3+store of the previous group runs here
        (between L1 and L2, where it is dependency-free), and this group's
        own L3 closure is returned via pend for the next slot."""
        x16, x8 = loads.pop(g, None) or g_load(g)
        h1 = hp.tile([P, 2, W * P], FP8, tag="h1")
        h2 = hp.tile([P, 2, W * P], FP8, tag="h2")
        g_layer(g, 0, lambda q: x8[:, :, 4 * q:4 * q + 4, :], h1)
        if prefetch is not None and prefetch not in loads:
            loads[prefetch] = g_load(prefetch)
        if pend is not None and pend["l3"] is not None:
            pend["l3"]()
        if mid1 is not None:
            mid1()
        g_layer(g, 1, lambda q: h1[:, :, q * 512:(q + 1) * 512], h2)
        if mid2 is not None:
            mid2()
        if pend is None:
            fT = g_layer(g, 2, lambda q: h2[:, :, q * 512:(q + 1) * 512], None)
            return g_store(g, fT, x16)

        def do_l3(g=g, h2=h2, x16=x16):
            fT = g_layer(g, 2, lambda q: h2[:, :, q * 512:(q + 1) * 512], None)
            g_store(g, fT, x16)
        pend["l3"] = do_l3
        return None

    # chain state: carry slices; coll tiles batch 4 output frames per DMA
    ch = {"prev": None, "coll": None}

    def chain_step(k):
        """out[:,100+k+1] = y + h*f(y); y is the transposed fp16 carry."""
        ytp = ch["prev"]   # [P, 2, P] fp16
        # L1 in fp16 straight off the carry (no cast hop): exact b1 seed
        # (K=1) + two kc-accumulated fp16 matmuls per mc chunk.
        c1 = chps.tile([P, 2, P], F32, tag="chp", name="c1")
        for mc in range(2):
            nc.tensor.matmul(c1[:, mc, :], b16[:, mc * P:(mc + 1) * P],
                             b16[:, D:D + P], start=True, stop=False)
        for mc in range(2):
            for kc in range(2):
                nc.tensor.matmul(c1[:, mc, :],
                                 w16[:, kc, mc * P:(mc + 1) * P],
                                 ytp[:, kc, :], start=False, stop=(kc == 1))
        h1c = hcp.tile([P, 2, P], FP8, tag="h1c")
        nc.scalar.activation(h1c[:], c1[:], AF.Tanh)
        c2 = chps.tile([P, 2, P], F32, tag="chp", name="c2")
        for mc in range(2):
            nc.tensor.matmul(c2[:, mc, :], bseed[:, :, 1, mc * P:(mc + 1) * P],
                             ones8[:], start=True, stop=False, perf_mode=DR)
        for mc in range(2):
            nc.tensor.matmul(c2[:, mc, :], w8[:, 1, :, mc * P:(mc + 1) * P],
                             h1c[:], start=False, stop=True, perf_mode=DR)
        h2c = hcp.tile([P, 2, P], FP8, tag="h2c")
        nc.scalar.activation(h2c[:], c2[:], AF.Tanh, scale=0.125)
        c3 = chps.tile([P, 2, P], F32, tag="chp", name="c3")
        for mc in range(2):
            nc.tensor.matmul(c3[:, mc, :], bseed[:, :, 2, mc * P:(mc + 1) * P],
                             ones8[:], start=True, stop=False, perf_mode=DR)
        for mc in range(2):
            nc.tensor.matmul(c3[:, mc, :], w8[:, 2, :, mc * P:(mc + 1) * P],
                             h2c[:], start=False, stop=True, perf_mode=DR)
        j = k % 4
        if j == 0:
            ch["coll"] = collp.tile([P, 2, 4, P], F32, tag="coll",
                                    name="coll")
        coll = ch["coll"]
        ytn = y16p.tile([P, 2, P], FP16, tag="ytn")
        nc.vector.scalar_tensor_tensor(ytn[:], c3[:], h8, ytp,
                                       ALU.mult, ALU.add)
        ch["prev"] = ytn
        nc.vector.scalar_tensor_tensor(coll[:, :, j, :], c3[:], h8, ytp,
                                       ALU.mult, ALU.add)
        if j == 3 or k == NSTEPS - 1:
            t0 = T_OBS + 1 + (k // 4) * 4
            nc.sync.dma_start(oTd[:, :, t0:t0 + j + 1, :], coll[:, :, 0:j + 1, :])

    # ---- schedule: PE warmup, last group first (chain dependency), then the
    # rest with chain steps embedded at the PE stall points (post-L1/post-L2,
    # where the tensor queue would otherwise wait on the Act engine) ----
    warm = chps.tile([P, 2, P], F32, tag="chp", name="warm")
    for i in range(10):
        nc.tensor.matmul(warm[:, i % 2, :], bseed[:, :, 0, 0:P], ones8[:],
                         start=True, stop=True, perf_mode=DR)
    wsink = hcp.tile([P, 2, P], FP8, tag="wsink")
    nc.scalar.activation(wsink[:], warm[:], AF.Tanh)

    oT_last = group(NG_FULL, prefetch=0)  # frames 96..99 -> out 97..100
    y0 = y16p.tile([P, 2, P], FP16, tag="y0")
    nc.vector.tensor_copy(y0[:], oT_last[:, :, W_LAST - 1, :])
    ch["prev"] = y0                       # y0 = out[:, 100] as fp16

    state = {"k": 0}

    def fill():
        if state["k"] < NSTEPS:
            chain_step(state["k"])
            state["k"] += 1

    pend = {"l3": None}
    for g in range(NG_FULL):
        group(g, mid1=fill, mid2=fill,
              prefetch=g + 1 if g + 1 < NG_FULL else None, pend=pend)
    pend["l3"]()
    while state["k"] < NSTEPS:
        fill()


def _build(h):
    import concourse.mybir as mybir
    import concourse.tile as tile
    from concourse import bacc

    F32 = mybir.dt.float32
    FP16 = mybir.dt.float16
    FP8 = mybir.dt.float8e4

    nc = bacc.Bacc("TRN2", target_bir_lowering=False, debug=False,
                   num_devices=NCORES)
    xT8d = nc.dram_tensor("xT8", [P, 2, T_OBS, P], FP8,
                          kind="ExternalInput").ap()
    xT16d = nc.dram_tensor("xT16", [P, 2, T_OBS, P], FP16,
                           kind="ExternalInput").ap()
    w8d = nc.dram_tensor("w8", [P, 3, 2, D], FP8, kind="ExternalInput").ap()
    bactd = nc.dram_tensor("bact", [P, 4], F32, kind="ExternalInput").ap()
    bseedd = nc.dram_tensor("bseed", [1, 2, 3, D], FP8,
                            kind="ExternalInput").ap()
    ones8d = nc.dram_tensor("ones8", [1, 2, P], FP8, kind="ExternalInput").ap()
    w16d = nc.dram_tensor("w16", [P, 2, D], FP16, kind="ExternalInput").ap()
    b16d = nc.dram_tensor("b16", [1, D + P], FP16, kind="ExternalInput").ap()
    oTd = nc.dram_tensor("oT", [P, 2, T, P], F32, kind="ExternalOutput").ap()

    with tile.TileContext(nc) as tc, ExitStack() as ctx:
        _emit(ctx, tc, xT8d, xT16d, w8d, bactd, bseedd, ones8d, w16d, b16d,
              oTd, h)
    nc.compile()
    return nc


def _host_inputs(inputs):
    """Shared (weights/bias) device arrays + h. Returns (h, shared)."""
    ts = np.asarray(inputs["time_steps"], np.float32)
    h = float(np.float32(ts[1]) - np.float32(ts[0]))

    f8 = ml_dtypes.float8_e4m3
    W1 = np.asarray(inputs["W1"], np.float32)
    W2 = np.asarray(inputs["W2"], np.float32)
    W3 = np.asarray(inputs["W3"], np.float32)
    b1 = np.asarray(inputs["b1"], np.float32)
    b2 = np.asarray(inputs["b2"], np.float32)
    b3 = np.asarray(inputs["b3"], np.float32)

    # w8[p, wi, kc, m] = 8*W_wi[kc*128+p, m]
    w8 = np.stack([8.0 * W1, 8.0 * W2, 8.0 * W3])  # [3, 256, 256]
    w8 = w8.reshape(3, 2, P, D).transpose(2, 0, 1, 3)  # [p, 3, kc, m]
    w8 = np.ascontiguousarray(w8).astype(f8)

    bact = np.stack([b1[:P], b1[P:], b2[:P], b2[P:]], axis=1)
    bact = np.ascontiguousarray(bact.astype(np.float32))

    # bseed[0, kc, wi, m] = 4*b_wi[m]  (K=2 DR ones contraction doubles it)
    bs = np.stack([4.0 * b1, 4.0 * b2, 4.0 * b3])  # [3, 256]
    bseed = np.broadcast_to(bs[None, None], (1, 2, 3, D))
    bseed = np.ascontiguousarray(bseed).astype(f8)
    ones8 = np.ones((1, 2, P), np.float32).astype(f8)

    # chain L1 runs in fp16: exact W1/b1 (unscaled) + a ones row
    w16 = W1.reshape(2, P, D).transpose(1, 0, 2)   # [p, kc, m]
    w16 = np.ascontiguousarray(w16).astype(np.float16)
    b16 = np.concatenate([b1, np.ones(P, np.float32)]).reshape(1, D + P)
    b16 = b16.astype(np.float16)

    shared = dict(w8=w8, bact=bact, bseed=bseed, ones8=ones8,
                  w16=w16, b16=b16)
    return h, shared


def make_in_maps(inputs):
    """Full per-core input maps (shared + per-core transposed latents)."""
    h, shared = _host_inputs(inputs)
    b3 = np.asarray(inputs["b3"], np.float32)
    lat = np.ascontiguousarray(np.asarray(inputs["latents"], np.float32))
    f8 = ml_dtypes.float8_e4m3

    in_maps = []
    for c in range(NCORES):
        lc = lat[c * PB:(c + 1) * PB]                  # [128b, 100t, 256d]
        xt = lc.transpose(2, 1, 0)                     # [256d, 100t, 128b]
        xt8 = xt.reshape(2, P, T_OBS, P).transpose(1, 0, 2, 3)
        xt16 = (xt + (np.float32(h) * b3)[:, None, None])
        xt16 = xt16.reshape(2, P, T_OBS, P).transpose(1, 0, 2, 3)
        m = dict(shared)
        m["xT8"] = np.ascontiguousarray(xt8).astype(f8)
        m["xT16"] = np.ascontiguousarray(xt16).astype(np.float16)
        in_maps.append(m)
    return h, in_maps


def assemble_out(inputs, core_outs):
    """De-transpose per-core oT outputs and patch the exact copy frames."""
    lat = np.asarray(inputs["latents"], np.float32)
    out = np.empty((B, T, D), np.float32)
    for c in range(NCORES):
        oT = core_outs[c]                              # [128p, 2dc, 120t, 128b]
        out[c * PB:(c + 1) * PB] = oT.transpose(3, 2, 1, 0).reshape(PB, T, D)
    out[:, 0, :] = lat[:, 0, :]
    out[:, 2, :] = lat[:, 1, :]
    return out


_CACHE = {}


def kernel(**inputs):
    from concourse.bass_utils import run_bass_kernel_spmd

    h, in_maps = make_in_maps(inputs)
    if h not in _CACHE:
        _CACHE[h] = _build(h)
    nc = _CACHE[h]

    res = run_bass_kernel_spmd(nc, in_maps, list(range(NCORES)))
    outs = [res.results[c]["oT"] for c in range(NCORES)]
    return assemble_out(inputs, outs)
